# revision 46
# baseline (speedup 1.0000x reference)
"""Trainium2 Bass kernel for a dense transformer block (B=4, T=2048, C=1024, H=16).

Sharding (8 cores): core c handles batch b=c//2 and head-group hg=c%2
(8 heads). Each core computes LN1 + QKV + causal attention for its 8 heads
over the full T=2048, then a pairwise ReduceScatter exchanges attnT halves
within each (batch) pair so core c finishes proj + LN2 + FFN for its own
T-half (rows hg*1024 .. hg*1024+1024) with the full set of 16 heads.

All activations on the matmul path are kept feature-major ("T in the free
dim") so every matmul runs with K=128 partition chunks and N=512/1024 moving
columns. Attention softmax weights, V, attnT, the RS exchange, Wproj and the
residual stream x2 are bf16; causal masking is applied post-exp on the
gpsimd engine (0/1 triangle multiply + memset), keeping the Activation
engine exclusively on Exp during attention. proj(t2=1) + LN2 of that half
are emitted interleaved into the attention tail so the PE fills its
Act-bound idle slots; the FFN runs single-pass over W1/W2 with N=1024
FFN1 matmuls.
"""

import sys
import numpy as np

for _p in ("/opt/trn_rl_repo",):
    if _p not in sys.path:
        sys.path.append(_p)

import concourse.bass as bass
import concourse.bacc as bacc
import concourse.tile as tile
import concourse.mybir as mybir

dt = mybir.dt
AF = mybir.ActivationFunctionType
ALU = mybir.AluOpType
F32 = dt.float32
F32R = dt.float32r
BF16 = dt.bfloat16

N_CORES = 8
B, T, C = 4, 2048, 1024
H, HS = 16, 64
HL = 8            # heads per core (local)
TH = T // 2       # t-half (rows per core for proj/FFN)
FF = 4 * C        # 4096
EPS = 1e-5

BF16_NP = dt.np(BF16)

_PROGRAM = None
NO_COLLECTIVE = False  # replace RS with local DMA (for TimelineSim)
USE_RANK1_BIAS = True
DEBUG_DUMP = False
STATIC_OFFSETS = False


def _build_program():
    nc = bacc.Bacc(
        "TRN2",
        target_bir_lowering=False,
        debug=False,
        num_devices=N_CORES,
        enable_partition_id=True,
    )

    # ---- I/O ----
    x_in = nc.dram_tensor("x", [T, C], F32, kind="ExternalInput")
    x_own_in = nc.dram_tensor("x_own", [TH, C], BF16, kind="ExternalInput")
    wq_in = nc.dram_tensor("wq", [C, HL * HS], BF16, kind="ExternalInput")
    wk_in = nc.dram_tensor("wk", [C, HL * HS], BF16, kind="ExternalInput")
    wv_in = nc.dram_tensor("wv", [C, HL * HS], BF16, kind="ExternalInput")
    wproj_in = nc.dram_tensor("wproj", [C, C], BF16, kind="ExternalInput")
    bproj_in = nc.dram_tensor("bproj_r", [1, C], BF16, kind="ExternalInput")
    w1_in = nc.dram_tensor("w1", [C, FF], BF16, kind="ExternalInput")
    b1c_in = nc.dram_tensor("b1c", [128, FF // 128], F32, kind="ExternalInput")
    w2_in = nc.dram_tensor("w2", [FF, C], BF16, kind="ExternalInput")
    b2r_in = nc.dram_tensor("b2r", [1, C], BF16, kind="ExternalInput")
    tri01_in = nc.dram_tensor("tri01", [128, 128], BF16, kind="ExternalInput")
    ident_in = nc.dram_tensor("ident", [128, 128], F32R, kind="ExternalInput")
    ident_bf_in = nc.dram_tensor("ident_bf", [128, 128], BF16, kind="ExternalInput")
    ones_row_in = nc.dram_tensor("ones_row", [1, 512], BF16, kind="ExternalInput")
    ones_c16_in = nc.dram_tensor("ones_c16", [128, HL], BF16, kind="ExternalInput")
    ones_bf_in = nc.dram_tensor("ones_bf", [1, 128], BF16, kind="ExternalInput")
    selc_in = nc.dram_tensor("selc", [128, 2], F32, kind="ExternalInput")
    sel2_in = nc.dram_tensor("sel2", [2, 128], F32R, kind="ExternalInput")
    out = nc.dram_tensor("out_half", [TH, C], F32, kind="ExternalOutput")
    if DEBUG_DUMP:
        dbg_attnT = nc.dram_tensor("dbg_attnT", [4, 128, T], BF16, kind="ExternalOutput")
        dbg_attnP = nc.dram_tensor("dbg_attnP", [128, 4, TH], BF16, kind="ExternalOutput")
        dbg_x2 = nc.dram_tensor("dbg_x2", [TH, C], BF16, kind="ExternalOutput")

    NT = T // 128          # 16 t-tiles (full T)
    NTH = TH // 128        # 8 t-tiles (own half)
    NC8 = C // 128         # 8 c-chunks
    NPAIR = HL // 2        # 4 head pairs
    NM = FF // 128         # 32 FFN m-tiles

    with tile.TileContext(nc) as tc:
        from contextlib import ExitStack

        ctx = ExitStack()
        with ctx:
            # ---------------- pools ----------------
            consts = ctx.enter_context(tc.tile_pool(name="consts", bufs=1))
            # PSUM: 2 (mm/transpose) + 4 (scores, reused by FFN1) + 2 (av)
            ps_mm = ctx.enter_context(tc.tile_pool(name="ps_mm", bufs=2, space="PSUM"))
            ps_sc = ctx.enter_context(tc.tile_pool(name="ps_sc", bufs=2, space="PSUM"))
            ps_av = ctx.enter_context(tc.tile_pool(name="ps_av", bufs=1, space="PSUM"))
            dram = ctx.enter_context(tc.tile_pool(name="dram", bufs=1, space="DRAM"))

            work = ctx.enter_context(tc.tile_pool(name="work", bufs=2, side="left"))
            ln_pool = ctx.enter_context(tc.tile_pool(name="ln", bufs=4, side="left"))
            exp_pool = ctx.enter_context(tc.tile_pool(name="exp", bufs=4, side="left"))
            attn_sm = ctx.enter_context(
                tc.tile_pool(name="attn_sm", bufs=2, side="left")
            )
            # phase-scoped pools:
            # right: poolA (dies after v) -> poolC/poolD (die after proj) +
            #        poolE (x2, lives to the end) + w1p/w2p (FFN)
            # left:  poolB (dies after attention) -> h1 (FFN), x2T
            ctxA = ExitStack()   # wq/wk/wv + xT       — dies after v production
            ctxB = ExitStack()   # kT, qT, v           — dies after attention
            ctxC = ExitStack()   # attnT, attnP        — dies after proj
            ctxD = ExitStack()   # wproj               — dies after proj
            poolFx = ctx.enter_context(tc.tile_pool(name="poolFx", bufs=1, side="left"))
            poolA = ctxA.enter_context(tc.tile_pool(name="poolA", bufs=1, side="right"))
            poolB = ctxB.enter_context(tc.tile_pool(name="poolB", bufs=1, side="left"))

            # ---------------- constants ----------------
            ident = consts.tile([128, 128], F32R, name="ident_t")
            nc.sync.dma_start(ident[:], ident_in[:])
            ident_bf = consts.tile([128, 128], BF16, name="ident_bf_t")
            nc.sync.dma_start(ident_bf[:], ident_bf_in[:])
            tri01 = consts.tile([128, 128], BF16, name="tri01_t")
            nc.sync.dma_start(tri01[:], tri01_in[:])
            ones_row = consts.tile([1, 512], BF16, name="ones_row_t")
            nc.sync.dma_start(ones_row[:], ones_row_in[:])
            ones_bf = consts.tile([1, 128], BF16, name="ones_bf_t")
            nc.sync.dma_start(ones_bf[:], ones_bf_in[:])
            bproj_r = consts.tile([1, C], BF16, name="bproj_t")
            nc.sync.dma_start(bproj_r[:], bproj_in[:])
            b1c = consts.tile([128, NM], F32, name="b1c_t")
            nc.sync.dma_start(b1c[:], b1c_in[:])
            b2r = consts.tile([1, C], BF16, name="b2r_t")
            nc.sync.dma_start(b2r[:], b2r_in[:])
            eps_t = consts.tile([128, 1], F32, name="eps_t")
            nc.vector.memset(eps_t[:], EPS)
            selc = consts.tile([128, 2], F32, name="selc_t")
            nc.sync.dma_start(selc[:], selc_in[:])
            sel2 = consts.tile([2, 128], F32R, name="sel2_t")
            nc.sync.dma_start(sel2[:], sel2_in[:])
            zero_c = consts.tile([128, 1], F32, name="zero_c")
            nc.vector.memset(zero_c[:], 0.0)

            # qkv weights, feature-major chunks: [128, 512] per c-chunk
            wq_t = [poolA.tile([128, HL * HS], BF16, name=f"wq_{i}") for i in range(NC8)]
            wk_t = [poolA.tile([128, HL * HS], BF16, name=f"wk_{i}") for i in range(NC8)]
            wv_t = [poolA.tile([128, HL * HS], BF16, name=f"wv_{i}") for i in range(NC8)]
            for i in range(NC8):
                nc.sync.dma_start(wq_t[i][:], wq_in[128 * i : 128 * i + 128, :])
                nc.sync.dma_start(wk_t[i][:], wk_in[128 * i : 128 * i + 128, :])
                nc.sync.dma_start(wv_t[i][:], wv_in[128 * i : 128 * i + 128, :])

            # ---------------- phase A: LN1 + transpose to x^T ----------------
            xT = poolA.tile([128, NC8, T], BF16, name="xT")  # x-hat transposed
            kT = [poolB.tile([128, T], BF16, name=f"kT_{p}") for p in range(NPAIR)]
            qT = [poolB.tile([128, T], BF16, name=f"qT_{p}") for p in range(NPAIR)]
            v_t = [poolB.tile([128, HL * 65], BF16, name=f"v_{i}") for i in range(NT)]
            for i in range(NT):
                x_t = work.tile([128, C], F32, tag="x_t")
                nc.sync.dma_start(x_t[:], x_in[128 * i : 128 * i + 128, :])
                st = ln_pool.tile([128, 2, 6], F32, tag="st")
                nc.vector.bn_stats(st[:, 0, :], x_t[:, 0:512])
                nc.vector.bn_stats(st[:, 1, :], x_t[:, 512:1024])
                mv = ln_pool.tile([128, 2], F32, tag="mv")
                nc.vector.bn_aggr(mv[:], st[:])
                rs = ln_pool.tile([128, 1], F32, tag="rs")
                nc.scalar.activation(rs[:], mv[:, 1:2], AF.Sqrt, bias=eps_t[:])
                nc.vector.reciprocal(rs[:], rs[:])
                xh = work.tile([128, C], F32R, tag="xh")
                nc.vector.tensor_scalar(
                    xh[:], x_t[:], mv[:, 0:1], rs[:], ALU.subtract, ALU.mult
                )
                for j in range(NC8):
                    ptr = ps_mm.tile([128, 128], F32R, tag="mm")
                    nc.tensor.transpose(ptr[:], xh[:, 128 * j : 128 * j + 128], ident[:])
                    nc.scalar.activation(
                        xT[:, j, 128 * i : 128 * i + 128], ptr[:], AF.Identity
                    )
                # V for this tile (keeps PE fed during the LN1 ramp)
                v3 = v_t[i].rearrange("p (h e) -> p h e", e=65)
                nc.sync.dma_start(
                    v3[:, :, 0:1], ones_c16_in[:, :].rearrange("p (h o) -> p h o", o=1)
                )
                ps = ps_mm.tile([128, 512], F32, tag="mm")
                for cc in range(NC8):
                    nc.tensor.matmul(
                        ps[:],
                        xT[:, cc, 128 * i : 128 * i + 128],
                        wv_t[cc][:],
                        start=(cc == 0),
                        stop=(cc == NC8 - 1),
                    )
                with nc.allow_low_precision(reason="bf16 v evict"):
                    nc.vector.tensor_copy(
                        v3[:, :, 1:65], ps[:].rearrange("p (h d) -> p h d", d=64)
                    )
                # k/q for each completed 512-col chunk
                if i % 4 == 3:
                    t4 = i // 4
                    for p in range(NPAIR):
                        for wt, dst in ((wk_t, kT), (wq_t, qT)):
                            ps = ps_mm.tile([128, 512], F32, tag="mm")
                            for cc in range(NC8):
                                nc.tensor.matmul(
                                    ps[:],
                                    wt[cc][:, 128 * p : 128 * p + 128],
                                    xT[:, cc, 512 * t4 : 512 * t4 + 512],
                                    start=(cc == 0),
                                    stop=(cc == NC8 - 1),
                                )
                            with nc.allow_low_precision(reason="bf16 kq evict"):
                                nc.vector.tensor_copy(
                                    dst[p][:, 512 * t4 : 512 * t4 + 512], ps[:]
                                )

            ctxA.close()  # wqkv + xT free after v is built

            # ---------------- phase C pools: attention + proj inputs ----------------
            # attnT [512, T] bf16; own T-half lives in cols [0:TH], peer in [TH:T]
            poolE = ctx.enter_context(tc.tile_pool(name="poolE", bufs=1, side="right"))
            w1_pool = ctx.enter_context(tc.tile_pool(name="w1p", bufs=3, side="right"))
            x_own = [poolE.tile([128, C], BF16, name=f"xo_{i}") for i in range(NTH)]
            for i in range(NTH):
                nc.sync.dma_start(x_own[i][:], x_own_in[128 * i : 128 * i + 128, :])
            x2 = x_own  # residual accumulated in place
            poolC = ctxC.enter_context(tc.tile_pool(name="poolC", bufs=1, side="right"))
            attnT = [poolC.tile([128, T], BF16, name=f"attnT_{p}") for p in range(NPAIR)]
            attnP = poolC.tile([128, NPAIR, TH], BF16, name="attnP")
            poolD = ctxD.enter_context(tc.tile_pool(name="poolD", bufs=1, side="right"))
            wp_t = [poolD.tile([128, C], BF16, name=f"wp_{i}") for i in range(NC8)]
            for i in range(NC8):
                nc.sync.dma_start(wp_t[i][:], wproj_in[128 * i : 128 * i + 128, :])
            x2T = poolFx.tile([128, NC8, TH], BF16, name="x2T")

            if STATIC_OFFSETS:
                tc_off = [512 * t4 for t4 in range(T // 512)]
            else:
                v_pid = nc.vector.partition_id()
                # local col offset for global t-chunk tc: (tc*512 + (pid%2)*1024) % 2048
                tc_off = [
                    ((v_pid % 2) * TH + (512 * t4)) % T for t4 in range(T // 512)
                ]

            rs_pool = ctxC.enter_context(tc.tile_pool(name="rs_dram", bufs=1, space="DRAM"))

            def rs_half(idx, lo):
                # ReduceScatter over pairs: exchange local peer cols [TH+lo : TH+lo+512]
                rs_in = rs_pool.tile([1024, 512], BF16, name=f"rs_in_{idx}")
                rs_out = rs_pool.tile([512, 512], BF16, name=f"rs_out_{idx}")
                for pp in range(NPAIR):
                    for sh in range(2):
                        tmp = work.tile([128, 512], BF16, tag="rs_tmp")
                        nc.vector.tensor_scalar(
                            tmp[:],
                            attnT[pp][:, TH + lo : TH + lo + 512],
                            selc[:, sh : sh + 1],
                            None,
                            ALU.mult,
                        )
                        nc.sync.dma_start(
                            rs_in[512 * sh + 128 * pp : 512 * sh + 128 * pp + 128, :],
                            tmp[:],
                        )
                if NO_COLLECTIVE:
                    nc.sync.dma_start(rs_out[:], rs_in[0:512, :])
                else:
                    nc.gpsimd.collective_compute(
                        "ReduceScatter",
                        ALU.add,
                        replica_groups=[[0, 1], [2, 3], [4, 5], [6, 7]],
                        ins=[rs_in[:]],
                        outs=[rs_out[:]],
                    )
                for pp in range(NPAIR):
                    nc.gpsimd.dma_start(
                        attnP[:, pp, lo : lo + 512],
                        rs_out[128 * pp : 128 * pp + 128, :],
                    )

            # ---------------- attention group ----------------
            def attn_group(t4, p):
                hA, hB = 2 * p, 2 * p + 1
                s_hi = 4 * (t4 + 1)
                avA = ps_av.tile([65, 512], F32, tag="avA")
                avB = ps_av.tile([65, 512], F32, tag="avB")
                for sb in range(s_hi):
                    psc = ps_sc.tile([128, 1024], F32, tag="sc")
                    nc.tensor.matmul(
                        psc[:, 0:512],
                        kT[p][0:64, 128 * sb : 128 * sb + 128],
                        qT[p][0:64, 512 * t4 : 512 * t4 + 512],
                        start=True,
                        stop=True,
                        tile_position=(0, 0),
                    )
                    nc.tensor.matmul(
                        psc[:, 512:1024],
                        kT[p][64:128, 128 * sb : 128 * sb + 128],
                        qT[p][64:128, 512 * t4 : 512 * t4 + 512],
                        start=True,
                        stop=True,
                        tile_position=(64, 0),
                    )
                    # causal masking is applied AFTER exp: the fully masked
                    # strip [0:w] is memset to 0 and the diagonal 128-col
                    # window is multiplied by a 0/1 triangle, both on the
                    # (otherwise idle) gpsimd engine. Exact zeros, like the
                    # reference's -inf.
                    j = sb - 4 * t4
                    w = 128 * j if j > 0 else 0
                    ee = exp_pool.tile([128, 1024], BF16, tag="ee")
                    if w > 0:
                        nc.scalar.activation(ee[:, w:512], psc[:, w:512], AF.Exp)
                        nc.scalar.activation(
                            ee[:, 512 + w : 1024], psc[:, 512 + w : 1024], AF.Exp
                        )
                    else:
                        nc.scalar.activation(ee[:], psc[:], AF.Exp)
                    if j >= 0:
                        with nc.allow_low_precision(reason="0/1 mask mult is exact"):
                            for hb in (0, 512):
                                if w > 0:
                                    nc.gpsimd.memset(ee[:, hb : hb + w], 0.0)
                                nc.gpsimd.tensor_tensor(
                                    ee[:, hb + w : hb + w + 128],
                                    ee[:, hb + w : hb + w + 128],
                                    tri01[:],
                                    ALU.mult,
                                )
                    nc.tensor.matmul(
                        avA[:],
                        v_t[sb][:, 65 * hA : 65 * hA + 65],
                        ee[:, 0:512],
                        start=(sb == 0),
                        stop=(sb == s_hi - 1),
                    )
                    nc.tensor.matmul(
                        avB[:],
                        v_t[sb][:, 65 * hB : 65 * hB + 65],
                        ee[:, 512:1024],
                        start=(sb == 0),
                        stop=(sb == s_hi - 1),
                    )
                esA = attn_sm.tile([65, 512], BF16, tag="esA")
                esB = attn_sm.tile([65, 512], BF16, tag="esB")
                with nc.allow_low_precision(reason="bf16 av evict"):
                    nc.vector.tensor_copy(esA[:], avA[:])
                    nc.vector.tensor_copy(esB[:], avB[:])
                # stack both denominator rows on partitions 0/1, one
                # reciprocal, then ONE K=2 matmul with a 0/1 selector
                # broadcasts 1/denomA to psum rows 0:64, 1/denomB to 64:128
                rAB = attn_sm.tile([2, 512], BF16, tag="rAB")
                nc.sync.dma_start(rAB[0:1, :], esA[0:1, :])
                nc.sync.dma_start(rAB[1:2, :], esB[0:1, :])
                rr = attn_sm.tile([2, 512], F32R, tag="rr")
                with nc.allow_low_precision(reason="softmax recip rounds to f32r"):
                    nc.vector.reciprocal(rr[:], rAB[:])
                rb_ps = ps_mm.tile([128, 512], F32, tag="mm")
                nc.tensor.matmul(rb_ps[:], sel2[:], rr[:], start=True, stop=True)
                av_sb = attn_sm.tile([128, 512], BF16, tag="av_sb")
                nc.sync.dma_start(av_sb[0:64, :], esA[1:65, :])
                nc.sync.dma_start(av_sb[64:128, :], esB[1:65, :])
                with nc.allow_low_precision(reason="bf16 attnT"):
                    nc.vector.tensor_tensor(
                        attnT[p][:, bass.ds(tc_off[t4], 512)],
                        av_sb[:],
                        rb_ps[:],
                        ALU.mult,
                    )

            # ---------------- proj chunk: one (t2, cpt) ----------------
            def proj_chunk(t2, cpt):
                ps = ps_mm.tile([128, 512], F32, tag="mm")
                for cc in range(NC8):
                    rhs = (
                        attnT[cc][:, 512 * t2 : 512 * t2 + 512]
                        if cc < NPAIR
                        else attnP[:, cc - NPAIR, 512 * t2 : 512 * t2 + 512]
                    )
                    nc.tensor.matmul(
                        ps[:],
                        wp_t[cc][:, 128 * cpt : 128 * cpt + 128],
                        rhs,
                        start=(cc == 0),
                        stop=(False if USE_RANK1_BIAS else cc == NC8 - 1),
                    )
                if USE_RANK1_BIAS:
                    nc.tensor.matmul(
                        ps[:],
                        bproj_r[0:1, 128 * cpt : 128 * cpt + 128],
                        ones_row[0:1, :],
                        start=False,
                        stop=True,
                    )
                pj_sb = work.tile([128, 512], BF16, tag="pj_sb")
                with nc.allow_low_precision(reason="bf16 proj evict"):
                    nc.vector.tensor_copy(pj_sb[:], ps[:])
                for tj in range(4):
                    tt = 4 * t2 + tj
                    ptr = ps_mm.tile([128, 128], BF16, tag="mm")
                    nc.tensor.transpose(
                        ptr[:], pj_sb[:, 128 * tj : 128 * tj + 128], ident_bf[:]
                    )
                    with nc.allow_low_precision(reason="bf16 residual"):
                        nc.vector.tensor_tensor(
                            x2[tt][:, 128 * cpt : 128 * cpt + 128],
                            ptr[:],
                            x_own[tt][:, 128 * cpt : 128 * cpt + 128],
                            ALU.add,
                        )

            # ---------------- LN2 of one T-half (batched rsqrt) ----------------
            def ln2_half(th):
                mv4 = ln_pool.tile([128, 4, 2], F32, tag="mv4")
                for k in range(4):
                    i = 4 * th + k
                    st = ln_pool.tile([128, 2, 6], F32, tag="st")
                    nc.vector.bn_stats(st[:, 0, :], x2[i][:, 0:512])
                    nc.vector.bn_stats(st[:, 1, :], x2[i][:, 512:1024])
                    nc.vector.bn_aggr(mv4[:, k, :], st[:])
                rs4 = ln_pool.tile([128, 4], F32, tag="rs4")
                nc.scalar.activation(rs4[:], mv4[:, :, 1], AF.Sqrt, bias=eps_t[:])
                nc.vector.reciprocal(rs4[:], rs4[:])
                for k in range(4):
                    i = 4 * th + k
                    xh = work.tile([128, C], F32R, tag="xh")
                    nc.vector.tensor_scalar(
                        xh[:], x2[i][:], mv4[:, k, 0:1], rs4[:, k : k + 1],
                        ALU.subtract, ALU.mult,
                    )
                    for j in range(NC8):
                        ptr = ps_mm.tile([128, 128], F32R, tag="mm")
                        nc.tensor.transpose(
                            ptr[:], xh[:, 128 * j : 128 * j + 128], ident[:]
                        )
                        with nc.allow_low_precision(reason="bf16 x2T evict"):
                            nc.vector.tensor_copy(
                                x2T[:, j, 128 * i : 128 * i + 128], ptr[:]
                            )

            # ---------------- emission: attention with proj/LN2 interleave ----
            for p in range(NPAIR):
                attn_group(3, p)
            for p in range(NPAIR):
                attn_group(1, p)
            # peer cols [TH+512:T] (even: t4=3, odd: t4=1) are complete
            rs_half(0, 512)
            for p in range(NPAIR):
                attn_group(2, p)
                # proj of own T-half chunk 1 interleaves into the Act-bound
                # attention tail (needs attnT cols 512:1024 + attnP from rs0)
                proj_chunk(1, 2 * p)
                proj_chunk(1, 2 * p + 1)
            ln2_half(1)
            for p in range(NPAIR):
                attn_group(0, p)
            # final RS half: local cols [TH : TH+512]
            rs_half(1, 0)
            for cpt in range(NC8):
                proj_chunk(0, cpt)
            ln2_half(0)

            if DEBUG_DUMP:
                for pp in range(NPAIR):
                    nc.sync.dma_start(dbg_attnT[pp], attnT[pp][:])
                nc.sync.dma_start(dbg_attnP[:], attnP[:])
                for i in range(NTH):
                    nc.sync.dma_start(dbg_x2[128 * i : 128 * i + 128, :], x2[i][:])

            # ---------------- phase G: FFN (single pass over W1/W2) ----------
            ctxD.close()
            ctxC.close()
            ctxB.close()
            h1_pool = ctx.enter_context(tc.tile_pool(name="h1p", bufs=1, side="left"))
            h1 = [h1_pool.tile([128, TH], BF16, name=f"h1_{m}") for m in range(NM)]
            w2_pool = ctx.enter_context(tc.tile_pool(name="w2p", bufs=1, side="right"))

            for m in range(NM):
                w1_m = w1_pool.tile([128, NC8, 128], BF16, tag="w1m")
                nc.sync.dma_start(
                    w1_m[:],
                    w1_in[:, 128 * m : 128 * m + 128].rearrange(
                        "(c p) m -> p c m", p=128
                    ),
                )
                ps = ps_sc.tile([128, 1024], F32, tag="sc")
                for cc in range(NC8):
                    # matmul N is capped at 512 by the single-PSUM-bank rule;
                    # two halves of the same 2-bank psum tile
                    nc.tensor.matmul(
                        ps[:, 0:512],
                        w1_m[:, cc, :],
                        x2T[:, cc, 0:512],
                        start=(cc == 0),
                        stop=(cc == NC8 - 1),
                    )
                    nc.tensor.matmul(
                        ps[:, 512:1024],
                        w1_m[:, cc, :],
                        x2T[:, cc, 512:1024],
                        start=(cc == 0),
                        stop=(cc == NC8 - 1),
                    )
                with nc.allow_low_precision(reason="bf16 h1 evict"):
                    nc.scalar.activation(
                        h1[m][:], ps[:], AF.Relu, bias=b1c[:, m : m + 1]
                    )

            for cp in range(2):
                w2h = [
                    w2_pool.tile([128, 512], BF16, tag=f"w2h_{m}", name=f"w2h_{cp}_{m}")
                    for m in range(NM)
                ]
                for m in range(NM):
                    nc.sync.dma_start(
                        w2h[m][:],
                        w2_in[128 * m : 128 * m + 128, 512 * cp : 512 * cp + 512],
                    )
                for tt in range(NTH):
                    ps = ps_mm.tile([128, 512], F32, tag="mm")
                    for m in range(NM):
                        nc.tensor.matmul(
                            ps[:],
                            h1[m][:, 128 * tt : 128 * tt + 128],
                            w2h[m][:],
                            start=(m == 0),
                            stop=(False if USE_RANK1_BIAS else m == NM - 1),
                        )
                    if USE_RANK1_BIAS:
                        nc.tensor.matmul(
                            ps[:],
                            ones_bf[0:1, :],
                            b2r[0:1, 512 * cp : 512 * cp + 512],
                            start=False,
                            stop=True,
                        )
                    out_sb = work.tile([128, 512], F32, tag="out_sb")
                    nc.vector.tensor_tensor(
                        out_sb[:], ps[:], x2[tt][:, 512 * cp : 512 * cp + 512], ALU.add
                    )
                    nc.sync.dma_start(
                        out[128 * tt : 128 * tt + 128, 512 * cp : 512 * cp + 512],
                        out_sb[:],
                    )

    nc.compile()
    return nc


def _get_program():
    global _PROGRAM
    if _PROGRAM is None:
        _PROGRAM = _build_program()
    return _PROGRAM


def make_in_maps(x, Wq, Wk, Wv, Wproj, bproj, ln1_g, ln1_b, ln2_g, ln2_b, W1, b1, W2, b2):
    """Host-side sharding: build the 8 per-core input maps."""
    x = np.asarray(x, np.float32)
    Wq = np.asarray(Wq, np.float32)
    Wk = np.asarray(Wk, np.float32)
    Wv = np.asarray(Wv, np.float32)
    Wproj = np.asarray(Wproj, np.float32)
    bproj = np.asarray(bproj, np.float32)
    ln1_g = np.asarray(ln1_g, np.float32)
    ln1_b = np.asarray(ln1_b, np.float32)
    ln2_g = np.asarray(ln2_g, np.float32)
    ln2_b = np.asarray(ln2_b, np.float32)
    W1 = np.asarray(W1, np.float32)
    b1 = np.asarray(b1, np.float32)
    W2 = np.asarray(W2, np.float32)
    b2 = np.asarray(b2, np.float32)

    assert np.all(ln1_b == 0.0) and np.all(ln2_b == 0.0), (
        "nonzero LN bias folding not implemented"
    )

    scale = 1.0 / np.sqrt(C)
    # [H, C, HS] -> g-folded, concat to [C, H*HS]
    Wq_f = (ln1_g[None, :, None] * Wq * scale).transpose(1, 0, 2).reshape(C, H * HS)
    Wk_f = (ln1_g[None, :, None] * Wk).transpose(1, 0, 2).reshape(C, H * HS)
    Wv_f = (ln1_g[None, :, None] * Wv).transpose(1, 0, 2).reshape(C, H * HS)
    W1_f = ln2_g[:, None] * W1

    # 0/1 causal triangle for the diagonal 128x128 sub-block (post-exp mult)
    s_idx = np.arange(128)[:, None]
    t_idx = np.arange(128)[None, :]
    tri01 = np.where(s_idx <= t_idx, 1.0, 0.0).astype(BF16_NP)

    sel2 = np.zeros((2, 128), np.float32)
    sel2[0, 0:64] = 1.0
    sel2[1, 64:128] = 1.0

    common = {
        "b1c": b1.reshape(FF // 128, 128).T.copy(),
        "b2r": b2.reshape(1, C).astype(BF16_NP),
        "tri01": tri01,
        "ident": np.eye(128, dtype=np.float32),
        "ident_bf": np.eye(128, dtype=np.float32).astype(BF16_NP),
        "ones_row": np.ones((1, 512), BF16_NP),
        "ones_c16": np.ones((128, HL), BF16_NP),
        "ones_bf": np.ones((1, 128), BF16_NP),
        "sel2": sel2,
        "bproj_r": bproj.reshape(1, C).astype(BF16_NP),
        "w1": W1_f.astype(BF16_NP),
        "w2": W2.astype(BF16_NP),
    }

    in_maps = []
    for c in range(N_CORES):
        b = c // 2
        hg = c % 2
        cols = slice(hg * HL * HS, (hg + 1) * HL * HS)
        # Wproj rows permuted: own head block first, then peer's
        own = Wproj[hg * HL * HS : (hg + 1) * HL * HS, :]
        peer = Wproj[(1 - hg) * HL * HS : (2 - hg) * HL * HS, :]
        selc = np.zeros((128, 2), np.float32)
        selc[:, 0] = hg
        selc[:, 1] = 1 - hg
        in_maps.append(
            dict(
                common,
                selc=selc,
                x=x[b],
                x_own=x[b, hg * TH : (hg + 1) * TH, :].astype(BF16_NP),
                wq=Wq_f[:, cols].astype(BF16_NP),
                wk=Wk_f[:, cols].astype(BF16_NP),
                wv=Wv_f[:, cols].astype(BF16_NP),
                wproj=np.concatenate([own, peer], axis=0).astype(BF16_NP),
            )
        )
    return in_maps


def assemble(results):
    out = np.empty((B, T, C), np.float32)
    for c in range(N_CORES):
        b, hg = c // 2, c % 2
        out[b, hg * TH : (hg + 1) * TH, :] = results[c]["out_half"]
    return out


def kernel(**inputs):
    from concourse import bass2jax

    nc = _get_program()
    in_maps = make_in_maps(**inputs)
    results = bass2jax.run_bass_via_pjrt(nc, in_maps, n_cores=N_CORES)
    return assemble(results)
